# revision 16
# baseline (speedup 1.0000x reference)
"""Distributed attention kernel for Trainium2 (8 NeuronCores).

Problem: B=2, N=2048, DIM=1024, 16 heads x 64 dims.
  qkv = hidden @ w_qkv ; per-head RMSNorm(q,k) ; RoPE ; softmax attention
  (bf16 matmuls, fp32 accumulation) ; out = attn @ w_out.

Sharding: tensor-parallel over heads. Core c owns heads {2c, 2c+1}.
Each core computes its 2 heads' attention plus its partial out-projection
(128 rows of w_out); host sums the 8 partial outputs (f32).

Per-core layout strategy (feature-major q/k, "S^T" scores):
  - Host passes hidden^T [1024, 4096] (bf16), so QKV matmuls produce
    q^T,k^T,v^T feature-major [128, 4096] directly (w slice as stationary).
  - RMSNorm over head dims (partition axis) via block-ones matmul (sum of
    squares broadcast to all 128 partitions) + exp(-0.5*ln(x)) on ScalarE
    (ACT Rsqrt/Reciprocal are banned; Ln+Exp share one table set).
  - RoPE: q*cos + rot_half(q)*sin, with rot_half folded into a constant
    128x128 permutation/sign matmul; sin table pre-permuted on host.
  - Scores computed transposed: S^T[k,q] tiles via lhsT=k^T chunk,
    rhs=q^T chunk; the two heads packed via PE row-tiling (K=64 each).
  - softmax without max subtraction (normed+roped logits are O(1)); exp on
    ScalarE reading 2 PSUM banks per instruction.
  - PV: stationary = [ones(64) | v(64)] so PSUM rows 0:64 get the softmax
    denominator (replicated) and rows 64:128 get unnormalized attn^T.
    1/denom = exp(-ln(denom)); normalization is one tensor_tensor mult.
  - out-proj: lhsT = attn^T (both heads = K=128), rhs = w_out rows slice.
"""

import os
import numpy as np
import ml_dtypes
from contextlib import ExitStack

import concourse.bass as bass
import concourse.mybir as mybir
import concourse.tile as tile
from concourse.vector_clock import VectorClock, ScopedClock
from concourse.bass_utils import run_bass_kernel_spmd


def _patched_drain_and_barrier(self, tick_clock, wait_clock):
    """Tile's exit drain puts one wait per live semaphore on a single
    instruction; this walrus build rejects >N waits per instruction
    ("Too many sync wait commands"). Split into one drain per proc."""
    nc = self.nc
    gc = tick_clock.global_clock
    n = len(gc)
    for p in range(n):
        if gc[p] > 0:
            vc = VectorClock([gc[p] if i == p else 0 for i in range(n)])
            d = nc.sync.drain()
            wait_clock.add_sem_waits(d.ins, ScopedClock({None: vc}))
    nc.all_engine_barrier()
    popped = nc._tile_sem_poison_stack.pop()
    assert popped is self._sem_poison
    nc.clear_and_free_semaphores(list(self.sems.allocated().values()))
    nc.all_engine_barrier()


tile.TileContext._drain_and_barrier = _patched_drain_and_barrier

_TPB_ENGINES = {
    mybir.EngineType.PE, mybir.EngineType.Activation, mybir.EngineType.Pool,
    mybir.EngineType.DVE, mybir.EngineType.SP,
}


def _fixup_wait_limits(nc):
    """Walrus here encodes at most 2 sync waits per instruction (1 for
    activations, which also get a table-load wait). Offload excess waits
    onto same-engine NoOps inserted immediately before the instruction."""
    import bass_rust

    f = nc.m.functions[0]
    for blk in f.blocks:
        insts = blk.instructions
        i = 0
        while i < len(insts):
            inst = insts[i]
            si = inst.sync_info
            waits = list(si.on_wait) if (si and si.on_wait) else []
            limit = 1
            if len(waits) > limit:
                if inst.engine not in _TPB_ENGINES:
                    i += 1
                    continue
                excess, keep = waits[:-limit], waits[-limit:]
                ups = list(si.on_update) if si.on_update else []
                inst.sync_info = mybir.SyncInfo(on_wait=keep, on_update=ups)
                pos = i
                for j in range(0, len(excess)):
                    ni = bass_rust.InstNoOp(
                        name=nc.get_next_instruction_name(), ins=[], outs=[])
                    ni.engine = inst.engine
                    ni.sync_info = mybir.SyncInfo(
                        on_wait=excess[j:j + 1], on_update=[])
                    nc.register_instruction(ni)
                    insts.insert(pos, ni)
                    pos += 1
                    i += 1
            i += 1

BF16 = mybir.dt.bfloat16
F32 = mybir.dt.float32
AF = mybir.ActivationFunctionType
ALU = mybir.AluOpType

DIM = 1024
NHEADS = 16
HD = 64
B = 2
N = 2048
T = B * N            # 4096 tokens
NCORES = 8
TC = 512             # token chunk (free dim of most tiles)
NTC = T // TC        # 8 token chunks
NQC = N // TC        # 4 q chunks per batch
KCH = 128            # k-position chunk (PV contraction)
NKC = N // KCH       # 16 k chunks per batch
EXPG = 2             # k-chunks exp'd per ACT instruction (2 psum banks)

_CACHE = {}


def build_graph():
    nc = bass.Bass()
    # register 1e-6 as a const AP so activation(bias=1e-6) lowers the same
    # way as the built-in 0.0/1.0 consts (tile-AP biases break walrus here)
    _epst = nc.alloc_sbuf_tensor("const-float32-eps", [128, 1], F32)
    nc.gpsimd.memset(_epst.ap(), 1e-6)
    nc.const_aps.aps[(F32, 1e-6)] = _epst.ap()
    nc.all_engine_barrier()
    ht = nc.declare_dram_parameter("ht", [DIM, T], BF16, isOutput=False)
    wq = nc.declare_dram_parameter("wq", [DIM, 128], BF16, isOutput=False)
    wk = nc.declare_dram_parameter("wk", [DIM, 128], BF16, isOutput=False)
    wv = nc.declare_dram_parameter("wv", [DIM, 128], BF16, isOutput=False)
    wo = nc.declare_dram_parameter("wo", [128, DIM], BF16, isOutput=False)
    cosw = nc.declare_dram_parameter("cosw", [128, N], BF16, isOutput=False)
    sinpw = nc.declare_dram_parameter("sinpw", [128, N], BF16, isOutput=False)
    rotm = nc.declare_dram_parameter("rotm", [128, 128], BF16, isOutput=False)
    ident = nc.declare_dram_parameter("ident", [128, 128], BF16, isOutput=False)
    ssqq = nc.declare_dram_parameter("ssqq", [128, 128], BF16, isOutput=False)
    ssqk = nc.declare_dram_parameter("ssqk", [128, 128], BF16, isOutput=False)
    outp = nc.declare_dram_parameter("out", [T, DIM], BF16, isOutput=True)

    with ExitStack() as ctx:
        tc_ = ctx.enter_context(tile.TileContext(nc))
        singles = ctx.enter_context(tc_.tile_pool(name="singles", bufs=1))
        work = ctx.enter_context(tc_.tile_pool(name="work", bufs=3))
        big = ctx.enter_context(tc_.tile_pool(name="big", bufs=1))

        # ---- load constants/weights ----
        ht_s = []
        for fc in range(8):
            t = singles.tile([128, T], BF16, tag=f"ht{fc}", name=f"ht{fc}")
            nc.sync.dma_start(out=t[:], in_=ht[fc * 128:(fc + 1) * 128, :])
            ht_s.append(t)
        w_s = {}
        for name, prm in (("wq", wq), ("wk", wk), ("wv", wv)):
            tiles = []
            for fc in range(8):
                t = singles.tile([128, 128], BF16, tag=f"{name}{fc}",
                                 name=f"{name}s{fc}")
                nc.sync.dma_start(out=t[:], in_=prm[fc * 128:(fc + 1) * 128, :])
                tiles.append(t)
            w_s[name] = tiles
        wo_s = singles.tile([128, DIM], BF16, tag="wo", name="wo_s")
        nc.sync.dma_start(out=wo_s[:], in_=wo[:, :])
        cos_s = singles.tile([128, N], BF16, tag="cos", name="cos_s")
        nc.sync.dma_start(out=cos_s[:], in_=cosw[:, :])
        sinp_s = singles.tile([128, N], BF16, tag="sinp", name="sinp_s")
        nc.sync.dma_start(out=sinp_s[:], in_=sinpw[:, :])
        rot_s = singles.tile([128, 128], BF16, tag="rot", name="rot_s")
        nc.sync.dma_start(out=rot_s[:], in_=rotm[:, :])
        id_s = singles.tile([128, 128], BF16, tag="id", name="id_s")
        nc.sync.dma_start(out=id_s[:], in_=ident[:, :])
        ssq_s = {}
        for name, prm in (("q", ssqq), ("k", ssqk)):
            t = singles.tile([128, 128], BF16, tag=f"ssq{name}",
                             name=f"ssq_s{name}")
            nc.sync.dma_start(out=t[:], in_=prm[:, :])
            ssq_s[name] = t

        # persistent activations
        qT = singles.tile([128, T], BF16, tag="qT", name="qT")
        kT = singles.tile([128, T], BF16, tag="kT", name="kT")
        vT = singles.tile([128, T], BF16, tag="vT", name="vT")
        # v token-major, per (batch, head): [ones(64) | v(64)] per 128-chunk
        v_sb = {}
        for b in range(B):
            for h in range(2):
                t = singles.tile([128, NKC * 128], BF16, tag=f"vsb{b}{h}",
                                 name=f"vsb{b}{h}")
                nc.vector.memset(t[:], 1.0)
                v_sb[(b, h)] = t

        # ---------------- phase 1: QKV + norm + rope ----------------
        p1_ctx = ExitStack()
        p1 = p1_ctx.enter_context(
            tc_.tile_pool(name="p1psum", bufs=1, space="PSUM"))
        for tcix in range(NTC):
            tsl = slice(tcix * TC, (tcix + 1) * TC)
            psl = _postbl(tsl)
            for which in ("q", "k", "v"):
                ps = p1.tile([128, TC], F32, tag="qkv", bufs=2,
                             name=f"ps_{which}{tcix}")
                for fc in range(8):
                    nc.tensor.matmul(
                        ps[:], w_s["w" + which][fc][:],
                        ht_s[fc][:, tsl], start=(fc == 0), stop=(fc == 7))
                if which == "v":
                    nc.vector.tensor_copy(vT[:, tsl], ps[:])
                    continue
                raw = work.tile([128, TC], BF16, tag="raw", name="raw")
                nc.vector.tensor_copy(raw[:], ps[:])
                sq = work.tile([128, TC], BF16, tag="sq", name="sq")
                nc.vector.tensor_tensor(sq[:], raw[:], raw[:], ALU.mult)
                ssqb = p1.tile([128, TC], F32, tag="ssqb", bufs=2,
                               name="ssqb")
                nc.tensor.matmul(ssqb[:], ssq_s[which][:], sq[:],
                                 start=True, stop=True)
                # scale = rsqrt(ssq/HD + eps) = exp(-0.5*ln(...))
                lnb = work.tile([128, TC], F32, tag="lnb", name="lnb")
                nc.scalar.activation(lnb[:], ssqb[:], AF.Ln,
                                     bias=1e-6, scale=1.0 / HD)
                scale = work.tile([128, TC], BF16, tag="scale", name="scale")
                nc.scalar.activation(scale[:], lnb[:], AF.Exp, scale=-0.5)
                qs = work.tile([128, TC], BF16, tag="qs", name="qs")
                nc.vector.tensor_tensor(qs[:], raw[:], scale[:], ALU.mult)
                # rope: qf = I @ (qs*cos) + R @ (qs*sin_perm)
                qs_cos = work.tile([128, TC], BF16, tag="qs_cos",
                                   name="qs_cos")
                nc.vector.tensor_tensor(qs_cos[:], qs[:], cos_s[:, psl],
                                        ALU.mult)
                qs_sin = work.tile([128, TC], BF16, tag="qs_sin",
                                   name="qs_sin")
                nc.vector.tensor_tensor(qs_sin[:], qs[:], sinp_s[:, psl],
                                        ALU.mult)
                qf = p1.tile([128, TC], F32, tag="qf", bufs=2, name="qf")
                nc.tensor.matmul(qf[:], id_s[:], qs_cos[:],
                                 start=True, stop=False)
                nc.tensor.matmul(qf[:], rot_s[:], qs_sin[:],
                                 start=False, stop=True)
                dst = qT if which == "q" else kT
                nc.vector.tensor_copy(dst[:, tsl], qf[:])

        # ------ phase 1.5: transpose v to token-major [ones|v] chunks ------
        for b in range(B):
            for kc in range(NKC):
                tsl = slice(b * N + kc * KCH, b * N + (kc + 1) * KCH)
                pt = p1.tile([128, 128], BF16, tag="vtp", bufs=2, name="vtp")
                nc.tensor.transpose(pt[:], vT[:, tsl], id_s[:])
                nc.vector.tensor_copy(
                    v_sb[(b, 0)][:, kc * 128 + 64: kc * 128 + 128],
                    pt[:, 0:64])
                nc.vector.tensor_copy(
                    v_sb[(b, 1)][:, kc * 128 + 64: kc * 128 + 128],
                    pt[:, 64:128])

        # ---------------- phase 2: attention + out-proj ----------------
        p1_ctx.close()
        p2 = ctx.enter_context(
            tc_.tile_pool(name="p2psum", bufs=1, space="PSUM"))
        inv_sqrt_hd = float(1.0 / np.sqrt(HD))
        for b in range(B):
            for qc in range(NQC):
                qsl = slice(b * N + qc * TC, b * N + (qc + 1) * TC)
                eA = big.tile([128, NKC * TC], BF16, tag="eA", bufs=1,
                              name="eA")
                eB = big.tile([128, NKC * TC], BF16, tag="eB", bufs=1,
                              name="eB")
                for kg in range(NKC // EXPG):
                    sA = p2.tile([128, EXPG * TC], F32, tag="sA", bufs=1,
                                 name="sA")
                    sB = p2.tile([128, EXPG * TC], F32, tag="sB", bufs=1,
                                 name="sB")
                    for j in range(EXPG):
                        kc = kg * EXPG + j
                        ksl = slice(b * N + kc * KCH, b * N + (kc + 1) * KCH)
                        nc.tensor.matmul(
                            sA[:, j * TC:(j + 1) * TC],
                            kT[0:64, ksl], qT[0:64, qsl],
                            start=True, stop=True, tile_position=(0, 0))
                        nc.tensor.matmul(
                            sB[:, j * TC:(j + 1) * TC],
                            kT[64:128, ksl], qT[64:128, qsl],
                            start=True, stop=True, tile_position=(64, 0))
                    esl = slice(kg * EXPG * TC, (kg + 1) * EXPG * TC)
                    nc.scalar.activation(eA[:, esl], sA[:], AF.Exp,
                                         scale=inv_sqrt_hd)
                    nc.scalar.activation(eB[:, esl], sB[:], AF.Exp,
                                         scale=inv_sqrt_hd)
                # PV with [ones|v]: rows 0:64 denom bcast, 64:128 attn^T
                pv = p2.tile([128, 2 * TC], F32, tag="pv", bufs=1, name="pv")
                for h, ebuf in ((0, eA), (1, eB)):
                    for kc in range(NKC):
                        nc.tensor.matmul(
                            pv[:, h * TC:(h + 1) * TC],
                            v_sb[(b, h)][:, kc * 128:(kc + 1) * 128],
                            ebuf[:, kc * TC:(kc + 1) * TC],
                            start=(kc == 0), stop=(kc == NKC - 1))
                lnd = work.tile([64, 2 * TC], F32, tag="lnd", name="lnd")
                nc.scalar.activation(lnd[:], pv[0:64, :], AF.Ln)
                rbc = work.tile([64, 2 * TC], F32, tag="rbc", name="rbc")
                nc.scalar.activation(rbc[:], lnd[:], AF.Exp, scale=-1.0)
                attnT = work.tile([128, TC], BF16, tag="attnT", name="attnT")
                nc.vector.tensor_tensor(
                    attnT[0:64, :], pv[64:128, 0:TC], rbc[0:64, 0:TC],
                    ALU.mult)
                nc.vector.tensor_tensor(
                    attnT[64:128, :], pv[64:128, TC:2 * TC],
                    rbc[0:64, TC:2 * TC], ALU.mult)
                # out-projection partial
                for mt in range(TC // 128):
                    for nn in range(DIM // TC):
                        po = p2.tile([128, TC], F32, tag="po", bufs=2,
                                     name="po")
                        nc.tensor.matmul(
                            po[:], attnT[:, mt * 128:(mt + 1) * 128],
                            wo_s[:, nn * TC:(nn + 1) * TC],
                            start=True, stop=True)
                        ob = work.tile([128, TC], BF16, tag="ob", name="ob")
                        nc.vector.tensor_copy(ob[:], po[:])
                        r0 = b * N + qc * TC + mt * 128
                        nc.sync.dma_start(
                            out=outp[r0:r0 + 128, nn * TC:(nn + 1) * TC],
                            in_=ob[:])
    _fixup_wait_limits(nc)
    return nc


def _postbl(tsl):
    """Map a token slice to the position slice in the [128, N] pos tables."""
    start, stop = tsl.start, tsl.stop
    return slice(start % N, (start % N) + (stop - start))


def _prep_inputs(hidden_states, cos, sin, w_qkv, norm_q_w, norm_k_w, w_out):
    bf = ml_dtypes.bfloat16
    hid = np.ascontiguousarray(
        np.asarray(hidden_states, np.float32).reshape(T, DIM).T).astype(bf)
    cosf = np.asarray(cos, np.float32)     # [N, 64]
    sinf = np.asarray(sin, np.float32)
    wqkv = np.asarray(w_qkv, np.float32)
    woutf = np.asarray(w_out, np.float32)
    wqn = np.asarray(norm_q_w, np.float32)
    wkn = np.asarray(norm_k_w, np.float32)

    # rot matrix lhsT: lhsT[j, d] = sigma(d) if j == pi(d) else 0
    rot = np.zeros((128, 128), np.float32)
    for d in range(128):
        dl = d % 64
        base = d - dl
        pi = base + (dl + 32) % 64
        sg = -1.0 if dl < 32 else 1.0
        rot[pi, d] = sg
    identm = np.eye(128, dtype=np.float32)

    # position tables, feature-major, stacked for 2 heads
    cosT = cosf.T                       # [64, N]
    sinpT = np.empty_like(sinf.T)       # sinp[j, n] = sin[n, (j+32)%64]
    for j in range(64):
        sinpT[j] = sinf[:, (j + 32) % 64]
    cosw = np.vstack([cosT, cosT]).astype(bf)
    sinpw = np.vstack([sinpT, sinpT]).astype(bf)

    # ssq lhsT: [k, m] = (head(k)==head(m)) / w(m)^2
    def ssq_mat(w):
        winv = np.where(np.abs(w) > 1e-20, 1.0 / (w * w), 0.0)
        m = np.zeros((128, 128), np.float32)
        for mm in range(128):
            h = mm // 64
            m[h * 64:(h + 1) * 64, mm] = winv[mm % 64]
        return m

    ssq_q = ssq_mat(wqn).astype(bf)
    ssq_k = ssq_mat(wkn).astype(bf)

    in_maps = []
    for c in range(NCORES):
        hA, hB = 2 * c, 2 * c + 1
        cols = np.r_[hA * HD:(hA + 1) * HD, hB * HD:(hB + 1) * HD]
        m = {
            "ht": hid,
            "wq": np.ascontiguousarray(wqkv[:, cols]).astype(bf),
            "wk": np.ascontiguousarray(wqkv[:, DIM + cols]).astype(bf),
            "wv": np.ascontiguousarray(wqkv[:, 2 * DIM + cols]).astype(bf),
            "wo": np.ascontiguousarray(
                woutf[c * 128:(c + 1) * 128, :]).astype(bf),
            "cosw": cosw,
            "sinpw": sinpw,
            "rotm": rot.astype(bf),
            "ident": identm.astype(bf),
            "ssqq": ssq_q,
            "ssqk": ssq_k,
        }
        in_maps.append(m)
    return in_maps


def kernel(hidden_states, cos, sin, w_qkv, norm_q_w, norm_k_w, w_out):
    if "nc" not in _CACHE:
        _CACHE["nc"] = build_graph()
    nc = _CACHE["nc"]
    in_maps = _prep_inputs(hidden_states, cos, sin, w_qkv, norm_q_w,
                           norm_k_w, w_out)
    trace = bool(int(os.environ.get("KERNEL_TRACE", "0")))
    res = run_bass_kernel_spmd(nc, in_maps, core_ids=list(range(NCORES)),
                               trace=trace)
    _CACHE["last_result"] = res
    outs = res.results
    total = np.zeros((T, DIM), np.float32)
    for m in outs:
        total += np.asarray(m["out"], dtype=np.float32)
    return total.reshape(B, N, DIM)
